# revision 1
# baseline (speedup 1.0000x reference)
"""GraphNorm Trainium2 kernel.

Problem: GraphNorm over N=500000 nodes, C=128 channels, B=512 graphs,
`batch` sorted. out = weight * (x - mean[batch]*mean_scale) / sqrt(var[batch]+eps) + bias
with per-graph mean/var of the mean_scale-centered features.

Strategy (8 cores, SPMD):
  - Graph-aligned data-parallel sharding over nodes (no graph straddles a
    core, so no cross-core reduction at all).
  - Per core, nodes are packed into chunks of 8192 (64 tiles of 128 nodes)
    aligned to graph boundaries; each chunk holds <= 32 graphs ("slots").
  - One pass over HBM: x is loaded once per chunk into SBUF, per-slot
    sums of [x | x^2] are computed with one-hot matmuls (A^T @ [x|x^2]
    accumulated in PSUM over the chunk), stats are turned into per-slot
    affine maps W = w*istd, B = b - mean*s*W, and the output
    out = x*W[slot] + B[slot] is produced by a one-hot gather matmul
    (A @ [W|B]) plus two vector ops, then stored.
  - var uses E[(x-s*m)^2] = E[x^2] - (2s - s^2) m^2 (exact identity).
  - Matmuls run as float32r (fast PE mode). The gather is split into
    hi+lo f32r matmuls accumulated in PSUM, recovering full f32 precision
    of the per-slot affine maps. Stats sums see only the f32r rounding of
    x (~1e-4 relative per element), which averages out over ~1000-node
    graphs (absolute mean/var error ~1e-5).
"""

import sys

sys.path.insert(0, "/opt/trn_rl_repo")

import numpy as np

import concourse.bass as bass
import concourse.bacc as bacc
import concourse.tile as tile
from concourse import mybir
from concourse.bass_utils import run_bass_kernel_spmd

f32 = mybir.dt.float32
f32r = mybir.dt.float32r
i32 = mybir.dt.int32

N, C, B = 500000, 128, 512
EPS = 1e-5
NCORES = 8
TPC = 64            # tiles per chunk
CHUNK = TPC * 128   # 8192 nodes per chunk
S = 64              # graph slots per chunk
GRP = 128 // S      # tiles per stacked A block (2)
NBLK = TPC // GRP   # stacked blocks per chunk (16)

_prog_cache = {}


def _build_program(nch):
    nc = bacc.Bacc()
    xin = nc.dram_tensor("xin", [nch * CHUNK, C], f32, kind="ExternalInput")
    bT = nc.dram_tensor("bT", [nch, 128, TPC], f32, kind="ExternalInput")
    invr = nc.dram_tensor("invr", [nch, 128, 1], f32, kind="ExternalInput")
    pb = nc.dram_tensor("pb", [128, 512], f32, kind="ExternalInput")
    outp = nc.dram_tensor("outp", [nch * CHUNK, C], f32, kind="ExternalOutput")

    with tile.TileContext(nc) as tc:
        with tc.tile_pool(name="const", bufs=1) as constp, \
             tc.tile_pool(name="dpool", bufs=2) as dpool, \
             tc.tile_pool(name="opool", bufs=2) as opool, \
             tc.tile_pool(name="btp", bufs=2) as btp, \
             tc.tile_pool(name="a4p", bufs=6) as a4p, \
             tc.tile_pool(name="at4p", bufs=2 * NBLK) as at4p, \
             tc.tile_pool(name="combp", bufs=4) as combp, \
             tc.tile_pool(name="statp", bufs=4) as statp, \
             tc.tile_pool(name="wbp", bufs=4) as wbp, \
             tc.tile_pool(name="pst_pool", bufs=2, space="PSUM") as pstp, \
             tc.tile_pool(name="atp_pool", bufs=2, space="PSUM") as atpp, \
             tc.tile_pool(name="pg_pool", bufs=4, space="PSUM") as pgp:

            # constants
            iota_sf = constp.tile([128, 128], f32)
            iota_si = constp.tile([128, 128], i32)
            nc.gpsimd.iota(iota_si, pattern=[[0, GRP], [1, S]], base=0,
                           channel_multiplier=0)
            nc.vector.tensor_copy(out=iota_sf, in_=iota_si)
            ident = constp.tile([128, 128], f32r)
            identi = constp.tile([128, 128], i32)
            nc.gpsimd.iota(identi, pattern=[[-1, 128]], base=127,
                           channel_multiplier=1)
            nc.vector.tensor_scalar(out=ident, in0=identi, scalar1=127,
                                    scalar2=None, op0=mybir.AluOpType.is_equal)
            pbt = constp.tile([128, 512], f32)
            nc.sync.dma_start(out=pbt, in_=pb[:, :])
            epst = constp.tile([128, 1], f32)
            nc.vector.memset(epst, EPS)

            for c in range(nch):
                # ---- loads
                D = dpool.tile([128, CHUNK], f32, tag="D")
                nc.sync.dma_start(
                    out=D.rearrange("p (t c) -> p t c", c=C),
                    in_=xin.ap()[c * CHUNK:(c + 1) * CHUNK, :]
                        .rearrange("(t p) c -> p t c", p=128))
                bTt = btp.tile([128, TPC], f32, tag="bT")
                nc.sync.dma_start(out=bTt, in_=bT.ap()[c])
                invt = btp.tile([128, 1], f32, tag="inv")
                nc.sync.dma_start(out=invt, in_=invr.ap()[c])

                # ---- one-hot blocks and their transposes
                A4s, AT4s = [], []
                for blk in range(NBLK):
                    A4 = a4p.tile([128, 128], f32r, tag="A4")
                    in0 = bass.AP(tensor=bTt.tensor,
                                  offset=bTt.offset + blk * GRP,
                                  ap=[bTt.ap[0], [1, GRP], [0, S]])
                    nc.vector.tensor_tensor(
                        out=A4.rearrange("p (g s) -> p g s", s=S),
                        in0=in0,
                        in1=iota_sf.rearrange("p (g s) -> p g s", s=S),
                        op=mybir.AluOpType.is_equal)
                    atp = atpp.tile([128, 128], f32r, tag="atp")
                    nc.tensor.transpose(atp, A4, ident)
                    AT4 = at4p.tile([128, 128], f32r, tag="AT4")
                    nc.scalar.copy(out=AT4, in_=atp)
                    A4s.append(A4)
                    AT4s.append(AT4)

                # ---- stats accumulation over the chunk
                pst = pstp.tile([S, 256], f32, tag="pst")
                for t in range(TPC):
                    blk, g = divmod(t, GRP)
                    comb = combp.tile([128, 256], f32r, tag="comb")
                    nc.gpsimd.tensor_copy(out=comb[:, 0:C],
                                          in_=D[:, t * C:(t + 1) * C])
                    nc.scalar.square(out=comb[:, C:2 * C],
                                     in_=D[:, t * C:(t + 1) * C])
                    nc.tensor.matmul(pst, lhsT=A4s[blk][:, g * S:(g + 1) * S],
                                     rhs=comb, start=(t == 0),
                                     stop=(t == TPC - 1))

                # ---- per-slot affine maps (replicated x2 along partitions)
                ps0 = statp.tile([S, 256], f32, tag="ps0")
                nc.vector.tensor_copy(out=ps0, in_=pst)
                pstR = statp.tile([128, 256], f32, tag="pstR")
                for g in range(GRP):
                    nc.sync.dma_start(out=pstR[g * S:(g + 1) * S, :], in_=ps0)
                mean = statp.tile([128, 128], f32, tag="mean")
                nc.vector.tensor_scalar_mul(out=mean, in0=pstR[:, 0:128],
                                            scalar1=invt)
                ex2 = statp.tile([128, 128], f32, tag="ex2")
                nc.vector.tensor_scalar_mul(out=ex2, in0=pstR[:, 128:256],
                                            scalar1=invt)
                wbx = wbp.tile([128, 256], f32, tag="wbx")
                W = wbx[:, 0:128]
                Bv = wbx[:, 128:256]
                var = statp.tile([128, 128], f32, tag="var")
                nc.vector.tensor_mul(out=var, in0=mean, in1=mean)
                nc.vector.tensor_mul(out=var, in0=var, in1=pbt[:, 0:128])
                nc.vector.tensor_sub(out=var, in0=ex2, in1=var)
                std = statp.tile([128, 128], f32, tag="std")
                nc.scalar.activation(out=std, in_=var,
                                     func=mybir.ActivationFunctionType.Sqrt,
                                     bias=epst, scale=1.0)
                nc.vector.reciprocal(out=std, in_=std)
                nc.vector.tensor_mul(out=W, in0=std, in1=pbt[:, 256:384])
                nc.vector.tensor_mul(out=mean, in0=mean, in1=pbt[:, 128:256])
                nc.vector.tensor_mul(out=mean, in0=mean, in1=W)
                nc.vector.tensor_sub(out=Bv, in0=pbt[:, 384:512], in1=mean)
                wb_hi = wbp.tile([128, 256], f32r, tag="wb_hi")
                nc.vector.tensor_copy(out=wb_hi, in_=wbx)
                wb_lo = wbp.tile([128, 256], f32r, tag="wb_lo")
                nc.vector.tensor_sub(out=wb_lo, in0=wbx, in1=wb_hi.bitcast(f32))

                # ---- gather + elementwise + store
                OUT = opool.tile([128, CHUNK], f32, tag="OUT")
                for t in range(TPC):
                    blk, g = divmod(t, GRP)
                    pg = pgp.tile([128, 256], f32, tag="pg")
                    at = AT4s[blk][g * S:(g + 1) * S, :]
                    nc.tensor.matmul(pg, lhsT=at, rhs=wb_hi[g * S:(g + 1) * S, :],
                                     start=True, stop=False)
                    nc.tensor.matmul(pg, lhsT=at, rhs=wb_lo[g * S:(g + 1) * S, :],
                                     start=False, stop=True)
                    osl = OUT[:, t * C:(t + 1) * C]
                    nc.vector.tensor_mul(out=osl, in0=D[:, t * C:(t + 1) * C],
                                         in1=pg[:, 0:128])
                    nc.vector.tensor_add(out=osl, in0=osl, in1=pg[:, 128:256])
                nc.sync.dma_start(
                    out=outp.ap()[c * CHUNK:(c + 1) * CHUNK, :]
                        .rearrange("(t p) c -> p t c", p=128),
                    in_=OUT.rearrange("p (t c) -> p t c", c=C))

    nc.finalize()
    return nc


def _shard(batch_np):
    """Graph-aligned sharding + chunk packing. Returns per-core metadata."""
    cnt = np.bincount(batch_np, minlength=B).astype(np.int64)
    cum = np.cumsum(cnt)  # cum[g] = nodes in graphs 0..g
    # split graphs into NCORES node-balanced contiguous ranges
    targets = (np.arange(1, NCORES) * (N / NCORES))
    bounds = np.searchsorted(cum, targets)  # graph index where each core ends
    gb = [0] + [int(b) + 1 for b in bounds] + [B]
    cores = []
    for i in range(NCORES):
        g0, g1 = gb[i], gb[i + 1]
        # pack graphs [g0,g1) into chunks of <= CHUNK nodes, <= S graphs
        chunks = []
        cur, cur_nodes = [], 0
        for g in range(g0, g1):
            n_g = int(cnt[g])
            if n_g == 0:
                continue
            assert n_g <= CHUNK, f"graph {g} has {n_g} nodes > chunk"
            if cur_nodes + n_g > CHUNK or len(cur) >= S:
                chunks.append((cur, cur_nodes))
                cur, cur_nodes = [], 0
            cur.append(g)
            cur_nodes += n_g
        if cur:
            chunks.append((cur, cur_nodes))
        node0 = int(cum[g0 - 1]) if g0 > 0 else 0
        cores.append({"g0": g0, "g1": g1, "node0": node0, "chunks": chunks})
    return cores, cnt


def kernel(x, batch, weight, bias, mean_scale, batch_size):
    x = np.ascontiguousarray(np.asarray(x, dtype=np.float32))
    batch_np = np.asarray(batch).astype(np.int64)
    weight = np.asarray(weight, dtype=np.float32)
    bias = np.asarray(bias, dtype=np.float32)
    ms = np.asarray(mean_scale, dtype=np.float32)

    cores, cnt = _shard(batch_np)
    nch = max(len(c["chunks"]) for c in cores)

    # param block: [coef | s | w | b], each [128] replicated to 128 partitions
    coef = 2.0 * ms - ms * ms
    pb_row = np.concatenate([coef, ms, weight, bias]).astype(np.float32)
    pb_np = np.ascontiguousarray(np.broadcast_to(pb_row, (128, 512)))

    in_maps = []
    metas = []
    for core in cores:
        x_pad = np.zeros((nch * CHUNK, C), np.float32)
        bT_np = np.zeros((nch, 128, TPC), np.float32)
        inv_np = np.zeros((nch, 128, 1), np.float32)
        pos = core["node0"]
        meta = []
        for ci, (graphs, n_nodes) in enumerate(core["chunks"]):
            x_pad[ci * CHUNK: ci * CHUNK + n_nodes] = x[pos: pos + n_nodes]
            slots = np.zeros(CHUNK, np.float32)
            off = 0
            for si, g in enumerate(graphs):
                n_g = int(cnt[g])
                slots[off: off + n_g] = si
                inv_np[ci, si::S, 0] = 1.0 / max(n_g, 1)
                off += n_g
            bT_np[ci] = slots.reshape(TPC, 128).T
            meta.append((pos, n_nodes))
            pos += n_nodes
        in_maps.append({"xin": x_pad, "bT": bT_np, "invr": inv_np, "pb": pb_np})
        metas.append(meta)

    if nch not in _prog_cache:
        _prog_cache[nch] = _build_program(nch)
    nc = _prog_cache[nch]

    import os
    trace = os.environ.get("GN_TRACE", "0") == "1"
    kw = {}
    if trace:
        kw = {"trace": True, "tmpdir": os.environ.get("GN_TRACE_DIR") or None}
    res = run_bass_kernel_spmd(nc, in_maps, core_ids=list(range(NCORES)), **kw)
    global last_results
    last_results = res

    out = np.empty((N, C), np.float32)
    for i in range(NCORES):
        op = res.results[i]["outp"]
        for ci, (pos, n_nodes) in enumerate(metas[i]):
            out[pos: pos + n_nodes] = op[ci * CHUNK: ci * CHUNK + n_nodes]
    return out

